# revision 6
# baseline (speedup 1.0000x reference)
"""BEV->Cylinder bilinear ring-sampling kernel for 8 Trainium2 NeuronCores.

Strategy (per core, 64 of the 512 (b,c) planes):
  * The 2048 sample points lie on a circle of radius 255.5 px; a tight
    rectangle cover of that ring (~17K px, tall-narrow rects on the steep
    arcs, short-wide on the shallow ones) is DMA'd into SBUF as
    X[plane, ring_px], rects split across the sync/scalar HWDGE queues and
    the gpsimd SWDGE queue, in circle order with the two ring halves
    interleaved.
  * The ring is split into two halves of equal chunk count living on SBUF
    partitions 0-63 / 64-127.  One PE transpose per 128-px column window
    transposes BOTH halves' chunks at once ([128,128] -> [128,128]).
  * col[plane, p] = sum_k w_k * I[corner_k(p)] accumulates in 4 PSUM
    quarter banks via one PE matmul per (chunk, output-eighth):
    col[:, win] += XT_chunk.T @ S_chunk_win with a host-precomputed
    sparse-in-dense weight matrix S.
  * As each output eighth's accumulation completes it is copied to SBUF and
    broadcast over the 64 z-bins by a single output DMA (2048-row, 1KB-row
    descriptors), eighths alternating between the two HWDGE queues so
    output write bandwidth is busy from ~15us onward.
All geometry/weights are input-independent compile-time constants baked into
the NEFF. Input dtype f32 is preserved end to end.
"""
import json
import math

import numpy as np

B, C, H_B, W_B = 4, 128, 512, 512
H_C, W_C = 64, 2048
MAX_RANGE = 50.0
XMIN, XMAX, YMIN, YMAX = -50.0, 50.0, -50.0, 50.0
NCORES = 8
PLANES = B * C // NCORES  # 64 planes per core

CHUNK = 128       # ring pixels per PE-transpose column window
NE = 8            # output eighths
QW = W_C // NE    # 256 columns per eighth
W_NARROW = 16     # row-merge width cap for narrow rects
SLACK = 6         # extra width allowed when merging rows
CLUSTER_GAP = 16  # split row runs when x-gap exceeds this

_CACHE = {}


# ----------------------------------------------------------------- geometry
def _sample_xy():
    """Sampling pixel coords exactly like the reference (jnp on CPU);
    numpy fallback differs only at ULP level."""
    try:
        import jax
        import jax.numpy as jnp
        cpu = jax.devices("cpu")[0]
        with jax.default_device(cpu):
            phi = jnp.linspace(-math.pi, math.pi, W_C)
            x_g = MAX_RANGE * jnp.cos(phi)
            y_g = MAX_RANGE * jnp.sin(phi)
            x = (x_g - XMIN) / (XMAX - XMIN) * (W_B - 1)
            y = (YMAX - y_g) / (YMAX - YMIN) * (H_B - 1)
            return np.asarray(x, np.float32), np.asarray(y, np.float32)
    except Exception:
        phi = np.linspace(-math.pi, math.pi, W_C, dtype=np.float32)
        x_g = (MAX_RANGE * np.cos(phi)).astype(np.float32)
        y_g = (MAX_RANGE * np.sin(phi)).astype(np.float32)
        x = ((x_g - XMIN) / (XMAX - XMIN) * (W_B - 1)).astype(np.float32)
        y = ((YMAX - y_g) / (YMAX - YMIN) * (H_B - 1)).astype(np.float32)
        return x, y


def _corners():
    x, y = _sample_xy()
    x0 = np.floor(x).astype(np.int64)
    y0 = np.floor(y).astype(np.int64)
    wx1 = (x - x0.astype(np.float32)).astype(np.float32)
    wx0 = (np.float32(1.0) - wx1).astype(np.float32)
    wy1 = (y - y0.astype(np.float32)).astype(np.float32)
    wy0 = (np.float32(1.0) - wy1).astype(np.float32)
    out = []
    for xi, wx in ((x0, wx0), (x0 + 1, wx1)):
        for yi, wy in ((y0, wy0), (y0 + 1, wy1)):
            w = (wx * wy).astype(np.float32)
            valid = (xi >= 0) & (xi < W_B) & (yi >= 0) & (yi < H_B)
            for p in range(W_C):
                if valid[p]:
                    out.append((p, int(yi[p]), int(xi[p]), float(w[p])))
    return out


def _build_cover(corner_list):
    """Tight rect cover of the ring: per-image-row x-runs merged vertically
    while the union width stays narrow. Steep arcs become tall-narrow rects,
    shallow arcs short-wide ones."""
    rows = {}
    for p, yy, xx, w in corner_list:
        rows.setdefault(yy, set()).add(xx)
    row_clusters = {}
    for yy, xs in rows.items():
        xs = sorted(xs)
        cl = [[xs[0], xs[0]]]
        for xx in xs[1:]:
            if xx - cl[-1][1] > CLUSTER_GAP:
                cl.append([xx, xx])
            else:
                cl[-1][1] = xx
        row_clusters[yy] = cl
    out = []
    open_rects = {}  # side -> [ya, yb, xa, xb, max_single_row_w]

    def flush(side):
        if side in open_rects:
            r = open_rects.pop(side)
            out.append((r[0], r[1] - r[0], r[2], r[3] - r[2] + 1))

    for yy in range(H_B):
        cl = row_clusters.get(yy)
        if cl is None:
            flush('L'); flush('R')
            continue
        if len(cl) == 1:
            flush('R')
            assign = [('L', cl[0])]
        else:
            assign = [('L', cl[0]), ('R', cl[-1])]
            for c in cl[1:-1]:
                out.append((yy, 1, c[0], c[1] - c[0] + 1))
        for side, (xa, xb) in assign:
            w = xb - xa + 1
            if side in open_rects:
                r = open_rects[side]
                nxa, nxb = min(r[2], xa), max(r[3], xb)
                lim = max(W_NARROW, int(1.15 * max(r[4], w)) + SLACK)
                if nxb - nxa + 1 <= lim:
                    r[1] = yy + 1
                    r[2], r[3] = nxa, nxb
                    r[4] = max(r[4], w)
                    continue
                flush(side)
            open_rects[side] = [yy, yy + 1, xa, xb, w]
    flush('L'); flush('R')
    return out


def build_plan():
    corner_list = _corners()
    rects = _build_cover(corner_list)
    # split tall rects so the half split balances and loads pipeline finer
    split_rects = []
    for (ya, h, xa, w) in rects:
        nparts = (h + 63) // 64
        step = (h + nparts - 1) // nparts
        for y in range(ya, ya + h, step):
            split_rects.append((y, min(step, ya + h - y), xa, w))
    rects = split_rects

    # order rects along the sampling circle so downstream chunks complete in
    # p order and output DMAs unblock early
    def _rect_p(rc):
        ya, h, xa, w = rc
        yc, xc = ya + h / 2.0, xa + w / 2.0
        phi = math.atan2(255.5 - yc, xc - 255.5)
        f = (phi + math.pi) / (2 * math.pi)
        # seam rects (p wraps 2047->0 at the left edge) must load FIRST so
        # eighth 0 can start early; eighth 7 also needs them but its last
        # matmul comes from its late-arc chunks anyway
        return f - 1.0 if f > 0.97 else f
    rects.sort(key=_rect_p)

    # split rect list into two halves (partitions 0-63 / 64-127) of equal
    # padded chunk count
    areas = [h * w for (ya, h, xa, w) in rects]
    total = sum(areas)
    cum, split = 0, len(rects)
    for i, a in enumerate(areas):
        cum += a
        if cum >= total / 2:
            split = i + 1
            break
    halves = [rects[:split], rects[split:]]
    rect_dmas = []   # (half, local_off, ya, h, xa, w) in placement order
    half_used = []
    pix2ring = {}    # (y, x) -> global ring position (half*hl + local)
    half_lists = [[], []]
    for hh, rl in enumerate(halves):
        off = 0
        for (ya, h, xa, w) in rl:
            half_lists[hh].append((hh, off, ya, h, xa, w))
            for r in range(h):
                for ccc in range(w):
                    key = (ya + r, xa + ccc)
                    if key not in pix2ring:
                        pix2ring[key] = (hh, off + r * w + ccc)
            off += h * w
        half_used.append(off)
    nchunk_h = (max(half_used) + CHUNK - 1) // CHUNK
    hl = nchunk_h * CHUNK
    # emission order: interleave the two halves so pair inputs arrive together
    order = []
    i0 = i1 = 0
    a0 = a1 = 0
    while i0 < len(half_lists[0]) or i1 < len(half_lists[1]):
        take0 = i1 >= len(half_lists[1]) or (i0 < len(half_lists[0]) and a0 <= a1)
        if take0:
            r = half_lists[0][i0]; i0 += 1; a0 += r[4] * r[5]
        else:
            r = half_lists[1][i1]; i1 += 1; a1 += r[4] * r[5]
        order.append(r)
    rect_dmas = order

    # matmuls: one per (half, chunk, eighth) with hits; interval = min..max p
    hits = {}
    for p, yy, xx, w in corner_list:
        hh, loc = pix2ring[(yy, xx)]
        c = loc // CHUNK
        d = hits.setdefault((hh, c, p // QW), {}).setdefault(p, {})
        r = loc % CHUNK
        d[r] = d.get(r, 0.0) + w

    # group by pair (chunk index), emission order: pair 0..nchunk_h-1
    plan_mms = []    # (pair, half, pst, plen, s_off) in emission order
    s_cols = []      # (hh, c, p, {row: w})
    s_off = 0
    for c in range(nchunk_h):
        for hh in (0, 1):
            for e in range(NE):
                key = (hh, c, e)
                if key not in hits:
                    continue
                ps = sorted(hits[key])
                pst, pen = ps[0], ps[-1]
                plen = pen - pst + 1
                plan_mms.append((c, hh, pst, plen, s_off))
                for p in range(pst, pen + 1):
                    s_cols.append((hh, c, p, hits[key].get(p, {})))
                s_off += plen
    S = np.zeros((CHUNK, s_off), dtype=np.float32)
    for j, (hh, c, p, rws) in enumerate(s_cols):
        for r, w in rws.items():
            S[r, j] += np.float32(w)

    # coverage check: every output column hit by at least one matmul
    covered = np.zeros(W_C, dtype=bool)
    for (c, hh, pst, plen, so) in plan_mms:
        covered[pst:pst + plen] = True
    assert covered.all(), "some output columns never written"

    # per-eighth last matmul (emission index) for staging triggers
    last_mm_of_eighth = {}
    for i, (c, hh, pst, plen, so) in enumerate(plan_mms):
        for e in range(pst // QW, (pst + plen - 1) // QW + 1):
            last_mm_of_eighth[e] = i

    return dict(rect_dmas=rect_dmas, half_used=half_used, hl=hl,
                nchunk_h=nchunk_h, S=S, plan_mms=plan_mms, sum_m=s_off,
                last_mm_of_eighth=last_mm_of_eighth)


# ------------------------------------------------------- walrus wait-split
def split_waits_json(bir, maxw=1):
    """This neuronxcc walrus accepts at most one sync-wait per instruction;
    move excess waits onto preceding wait-only EventSemaphore ops."""
    uid = [0]
    for fn in bir["functions"]:
        for blk in fn["blocks"]:
            out = []
            for inst in blk["instructions"]:
                si = inst.get("sync_info")
                if si and si.get("on_wait") and len(si["on_wait"]) > maxw:
                    waits = si["on_wait"]
                    extra, keep = waits[:-maxw], waits[-maxw:]
                    for i in range(0, len(extra), maxw):
                        uid[0] += 1
                        out.append({
                            "debug": inst.get("debug", 0),
                            "engine": inst["engine"],
                            "ins": [],
                            "name": f"I-ws-{uid[0]}",
                            "opcode": "EventSemaphore",
                            "outs": [],
                            "sync_info": {"on_update": [],
                                          "on_wait": extra[i:i + maxw]},
                        })
                    si["on_wait"] = keep
                out.append(inst)
            blk["instructions"] = out
    return bir


# ------------------------------------------------------------ device build
def build_nc(plan, repeat=1):
    import concourse.bass as bass
    import concourse.mybir as mybir
    from concourse.tile import TileContext

    class PatchedBass(bass.Bass):
        def to_json_bytes(self):
            data = json.loads(super().to_json_bytes())
            return json.dumps(split_waits_json(data, 1)).encode()

    nc = PatchedBass()
    x_in = nc.dram_tensor("bev", [PLANES, H_B, W_B], mybir.dt.float32,
                          kind="ExternalInput")
    # unused input whose shape varies with `repeat`: defeats the NEFF cache's
    # shape-only HLO hash so timing variants compile separately
    nc.dram_tensor("nonce", [1, max(1, repeat)], mybir.dt.float32,
                   kind="ExternalInput")
    out = nc.dram_tensor("out", [PLANES, H_C, W_C], mybir.dt.float32,
                         kind="ExternalOutput")
    s_const = nc.inline_tensor(plan["S"], name="s_const")
    ident = nc.inline_tensor(np.eye(128, dtype=np.float32), name="ident")

    hl = plan["hl"]
    nch = plan["nchunk_h"]

    with TileContext(nc) as tc:
        with tc.tile_pool(name="sb", bufs=1) as pool, \
             tc.tile_pool(name="pst", bufs=4, space="PSUM") as pst, \
             tc.tile_pool(name="psc", bufs=1, space="PSUM") as psc:
            x_sb = pool.tile([128, hl], mybir.dt.float32)
            xt_sb = pool.tile([128, nch * CHUNK], mybir.dt.float32)
            s_sb = pool.tile([CHUNK, plan["sum_m"]], mybir.dt.float32)
            id_sb = pool.tile([128, 128], mybir.dt.float32)
            col_sb = pool.tile([PLANES, W_C], mybir.dt.float32)

            nc.gpsimd.dma_start(s_sb[:], s_const[:])
            nc.gpsimd.dma_start(id_sb[:], ident[:])

            for _rep in range(repeat):
                # pad/garbage ring tails must read as zeros
                for hh in (0, 1):
                    used = plan["half_used"][hh]
                    if hl > used:
                        nc.vector.memset(
                            x_sb[64 * hh:64 * hh + 64, used:hl], 0.0)

                # ring cover loads split across both HWDGE queues + SWDGE;
                # greedy least-loaded assignment (SWDGE descriptor generation
                # on gpsimd costs 1us + 0.34ns/descriptor, HWDGE ~0.6us flat)
                qload = {"sync": 0.0, "scalar": 0.0, "gpsimd": 0.0}
                qeng = {"sync": nc.sync, "scalar": nc.scalar,
                        "gpsimd": nc.gpsimd}
                for (hh, off, ya, h, xa, w) in plan["rect_dmas"]:
                    cost = {"sync": 600.0, "scalar": 600.0,
                            "gpsimd": 1000.0 + 0.34 * 64 * h}
                    qn = min(qload, key=lambda q: qload[q] + cost[q])
                    qload[qn] += cost[qn]
                    qeng[qn].dma_start(
                        x_sb[64 * hh:64 * hh + 64, off:off + h * w],
                        x_in[:, ya:ya + h, xa:xa + w])

                # output eighth accumulators: 4 PSUM banks (one per quarter)
                col_q = [psc.tile([PLANES, 2 * QW], mybir.dt.float32,
                                  name=f"colq{q}", tag=f"colq{q}")
                         for q in range(NE // 2)]
                for q in range(NE // 2):
                    nc.vector.memset(col_q[q][:], 0.0)

                def col_win(pst_, plen):
                    q, lo = pst_ // (2 * QW), pst_ % (2 * QW)
                    return col_q[q][:, lo:lo + plen]

                # per-pair: one [128,128] transpose covers one chunk of each
                # half; copyback to SBUF; then that pair's matmuls; stage +
                # output DMA fire as eighths complete
                mms = plan["plan_mms"]
                mmi = 0
                stage_by_mm = {}
                for e, i in plan["last_mm_of_eighth"].items():
                    stage_by_mm.setdefault(i, []).append(e)
                out_queues = [nc.sync, nc.scalar]
                eighth_emitted = [False] * NE
                nout = 0
                emitted_t = 0

                def emit_transpose(c):
                    xt_ps = pst.tile([128, 128], mybir.dt.float32,
                                     name=f"xtps{c}", tag="xtps")
                    nc.tensor.transpose(
                        xt_ps[:], x_sb[:, CHUNK * c:CHUNK * (c + 1)], id_sb[:])
                    nc.vector.tensor_copy(
                        xt_sb[:, CHUNK * c:CHUNK * (c + 1)], xt_ps[:])

                for c in range(nch):
                    # keep 2-3 transposes in flight ahead of this pair's
                    # matmuls so PE never stalls on the DVE copyback
                    while emitted_t < min(c + 3, nch):
                        emit_transpose(emitted_t)
                        emitted_t += 1
                    while mmi < len(mms) and mms[mmi][0] == c:
                        _, hh, pst_, plen, so = mms[mmi]
                        nc.tensor.matmul(
                            col_win(pst_, plen),
                            xt_sb[:, CHUNK * c + 64 * hh:CHUNK * c + 64 * hh + 64],
                            s_sb[:, so:so + plen],
                            start=False, stop=False, skip_group_check=True)
                        for e in stage_by_mm.get(mmi, []):
                            # stage eighth e to SBUF and write it out
                            ecols = slice(QW * e, QW * (e + 1))
                            q, lo = e // 2, (e % 2) * QW
                            nc.vector.tensor_copy(
                                col_sb[:, ecols], col_q[q][:, lo:lo + QW])
                            srcb = col_sb[:, None, ecols].to_broadcast(
                                (PLANES, H_C, QW))
                            dst = bass.AP(out, QW * e, [
                                [H_C * W_C, PLANES],
                                [W_C, H_C],
                                [1, QW],
                            ])
                            out_queues[nout % 2].dma_start(dst, srcb)
                            nout += 1
                            eighth_emitted[e] = True
                        mmi += 1
                assert mmi == len(mms) and all(eighth_emitted)
    return nc


# ------------------------------------------------------------------ runner
def _get_state():
    if "state" in _CACHE:
        return _CACHE["state"]
    import jax
    import concourse.mybir as mybir
    from concourse import bass2jax
    from jax.sharding import Mesh, PartitionSpec
    from jax.experimental.shard_map import shard_map

    plan = build_plan()
    nc = build_nc(plan)
    bass2jax.install_neuronx_cc_hook()

    partition_name = (nc.partition_id_tensor.name
                      if nc.partition_id_tensor else None)
    in_names, out_names, out_avals, zero_outs = [], [], [], []
    for alloc in nc.m.functions[0].allocations:
        if not isinstance(alloc, mybir.MemoryLocationSet):
            continue
        name = alloc.memorylocations[0].name
        if alloc.kind == "ExternalInput":
            if name != partition_name:
                in_names.append(name)
        elif alloc.kind == "ExternalOutput":
            shape = tuple(alloc.tensor_shape)
            dtype = mybir.dt.np(alloc.dtype)
            out_names.append(name)
            out_avals.append(jax.core.ShapedArray(shape, dtype))
            zero_outs.append(np.zeros(shape, dtype))
    n_params = len(in_names)
    n_outs = len(out_names)
    all_names = in_names + out_names
    if partition_name is not None:
        all_names = all_names + [partition_name]
    donate = tuple(range(n_params, n_params + n_outs))

    def _body(*args):
        operands = list(args)
        if partition_name is not None:
            operands.append(bass2jax.partition_id_tensor())
        outs = bass2jax._bass_exec_p.bind(
            *operands,
            out_avals=tuple(out_avals),
            in_names=tuple(all_names),
            out_names=tuple(out_names),
            lowering_input_output_aliases=(),
            sim_require_finite=True,
            sim_require_nnan=True,
            nc=nc,
        )
        return tuple(outs)

    devices = jax.devices()[:NCORES]
    mesh = Mesh(np.asarray(devices), ("core",))
    specs = (PartitionSpec("core"),) * (n_params + n_outs)
    out_specs = (PartitionSpec("core"),) * n_outs
    fn = jax.jit(
        shard_map(_body, mesh=mesh, in_specs=specs, out_specs=out_specs,
                  check_rep=False),
        donate_argnums=donate, keep_unused=True)

    nonce = np.zeros((NCORES, 1), np.float32)
    state = dict(fn=fn, zero_outs=zero_outs, prev=None, nc=nc, plan=plan,
                 nonce=nonce)
    _CACHE["state"] = state
    return state


def kernel(bev_feat):
    bev = np.ascontiguousarray(np.asarray(bev_feat, dtype=np.float32))
    st = _get_state()
    global_in = bev.reshape(B * C, H_B, W_B)  # cores split axis 0: 64 each
    if st["prev"] is not None:
        zouts = st["prev"]          # donate previous device outputs
    else:
        zouts = [np.zeros((NCORES * z.shape[0], *z.shape[1:]), z.dtype)
                 for z in st["zero_outs"]]
    outs = st["fn"](global_in, st["nonce"], *zouts)
    result = np.asarray(outs[0])    # [512, 64, 2048]
    st["prev"] = list(outs)
    return result.reshape(B, C, H_C, W_C)


# revision 10
# speedup vs baseline: 1.4906x; 1.4906x over previous
"""BEV->Cylinder bilinear ring-sampling kernel for 8 Trainium2 NeuronCores.

Strategy (per core, 64 of the 512 (b,c) planes):
  * The 2048 sample points lie on a circle of radius 255.5 px; a tight
    rectangle cover of that ring (~17K px, tall-narrow rects on the steep
    arcs, short-wide on the shallow ones) is DMA'd into SBUF as
    X[plane, ring_px], rects split across the sync/scalar HWDGE queues and
    the gpsimd SWDGE queue, in circle order with the two ring halves
    interleaved.
  * The ring is split into two halves of equal chunk count living on SBUF
    partitions 0-63 / 64-127.  One PE transpose per 128-px column window
    transposes BOTH halves' chunks at once ([128,128] -> [128,128]).
  * col[plane, p] = sum_k w_k * I[corner_k(p)] accumulates in 4 PSUM
    quarter banks via one PE matmul per (chunk, output-eighth):
    col[:, win] += XT_chunk.T @ S_chunk_win with a host-precomputed
    sparse-in-dense weight matrix S.
  * As each output eighth's accumulation completes it is copied to SBUF and
    broadcast over the 64 z-bins by a single output DMA (2048-row, 1KB-row
    descriptors), eighths alternating between the two HWDGE queues so
    output write bandwidth is busy from ~15us onward.
All geometry/weights are input-independent compile-time constants baked into
the NEFF. Input dtype f32 is preserved end to end.
"""
import json
import math

import numpy as np

B, C, H_B, W_B = 4, 128, 512, 512
H_C, W_C = 64, 2048
MAX_RANGE = 50.0
XMIN, XMAX, YMIN, YMAX = -50.0, 50.0, -50.0, 50.0
NCORES = 8
PLANES = B * C // NCORES  # 64 planes per core

CHUNK = 128       # ring pixels per PE-transpose column window
NQ = 4            # output quarters (2KB DMA rows — the output sweet spot)
QW = W_C // NQ    # 512 columns per quarter
W_NARROW = 16     # row-merge width cap for narrow rects
SLACK = 6         # extra width allowed when merging rows
CLUSTER_GAP = 16  # split row runs when x-gap exceeds this

_CACHE = {}


# ----------------------------------------------------------------- geometry
def _sample_xy():
    """Sampling pixel coords exactly like the reference (jnp on CPU);
    numpy fallback differs only at ULP level."""
    try:
        import jax
        import jax.numpy as jnp
        cpu = jax.devices("cpu")[0]
        with jax.default_device(cpu):
            phi = jnp.linspace(-math.pi, math.pi, W_C)
            x_g = MAX_RANGE * jnp.cos(phi)
            y_g = MAX_RANGE * jnp.sin(phi)
            x = (x_g - XMIN) / (XMAX - XMIN) * (W_B - 1)
            y = (YMAX - y_g) / (YMAX - YMIN) * (H_B - 1)
            return np.asarray(x, np.float32), np.asarray(y, np.float32)
    except Exception:
        phi = np.linspace(-math.pi, math.pi, W_C, dtype=np.float32)
        x_g = (MAX_RANGE * np.cos(phi)).astype(np.float32)
        y_g = (MAX_RANGE * np.sin(phi)).astype(np.float32)
        x = ((x_g - XMIN) / (XMAX - XMIN) * (W_B - 1)).astype(np.float32)
        y = ((YMAX - y_g) / (YMAX - YMIN) * (H_B - 1)).astype(np.float32)
        return x, y


def _corners():
    x, y = _sample_xy()
    x0 = np.floor(x).astype(np.int64)
    y0 = np.floor(y).astype(np.int64)
    wx1 = (x - x0.astype(np.float32)).astype(np.float32)
    wx0 = (np.float32(1.0) - wx1).astype(np.float32)
    wy1 = (y - y0.astype(np.float32)).astype(np.float32)
    wy0 = (np.float32(1.0) - wy1).astype(np.float32)
    out = []
    for xi, wx in ((x0, wx0), (x0 + 1, wx1)):
        for yi, wy in ((y0, wy0), (y0 + 1, wy1)):
            w = (wx * wy).astype(np.float32)
            valid = (xi >= 0) & (xi < W_B) & (yi >= 0) & (yi < H_B)
            for p in range(W_C):
                if valid[p]:
                    out.append((p, int(yi[p]), int(xi[p]), float(w[p])))
    return out


def _build_cover(corner_list):
    """Tight rect cover of the ring: per-image-row x-runs merged vertically
    while the union width stays narrow. Steep arcs become tall-narrow rects,
    shallow arcs short-wide ones."""
    rows = {}
    for p, yy, xx, w in corner_list:
        rows.setdefault(yy, set()).add(xx)
    row_clusters = {}
    for yy, xs in rows.items():
        xs = sorted(xs)
        cl = [[xs[0], xs[0]]]
        for xx in xs[1:]:
            if xx - cl[-1][1] > CLUSTER_GAP:
                cl.append([xx, xx])
            else:
                cl[-1][1] = xx
        row_clusters[yy] = cl
    out = []
    open_rects = {}  # side -> [ya, yb, xa, xb, max_single_row_w]

    def flush(side):
        if side in open_rects:
            r = open_rects.pop(side)
            out.append((r[0], r[1] - r[0], r[2], r[3] - r[2] + 1))

    for yy in range(H_B):
        cl = row_clusters.get(yy)
        if cl is None:
            flush('L'); flush('R')
            continue
        if len(cl) == 1:
            flush('R')
            assign = [('L', cl[0])]
        else:
            assign = [('L', cl[0]), ('R', cl[-1])]
            for c in cl[1:-1]:
                out.append((yy, 1, c[0], c[1] - c[0] + 1))
        for side, (xa, xb) in assign:
            w = xb - xa + 1
            if side in open_rects:
                r = open_rects[side]
                nxa, nxb = min(r[2], xa), max(r[3], xb)
                lim = max(W_NARROW, int(1.15 * max(r[4], w)) + SLACK)
                if nxb - nxa + 1 <= lim:
                    r[1] = yy + 1
                    r[2], r[3] = nxa, nxb
                    r[4] = max(r[4], w)
                    continue
                flush(side)
            open_rects[side] = [yy, yy + 1, xa, xb, w]
    flush('L'); flush('R')
    return out


def build_plan():
    corner_list = _corners()
    rects = _build_cover(corner_list)
    # split tall rects so the half split balances and loads pipeline finer
    split_rects = []
    for (ya, h, xa, w) in rects:
        nparts = (h + 63) // 64
        step = (h + nparts - 1) // nparts
        for y in range(ya, ya + h, step):
            split_rects.append((y, min(step, ya + h - y), xa, w))
    rects = split_rects

    # order rects along the sampling circle so downstream chunks complete in
    # p order and output DMAs unblock early
    def _rect_p(rc):
        ya, h, xa, w = rc
        yc, xc = ya + h / 2.0, xa + w / 2.0
        phi = math.atan2(255.5 - yc, xc - 255.5)
        f = (phi + math.pi) / (2 * math.pi)
        # seam rects (p wraps 2047->0 at the left edge) must load FIRST so
        # eighth 0 can start early; eighth 7 also needs them but its last
        # matmul comes from its late-arc chunks anyway
        return f - 1.0 if f > 0.97 else f
    rects.sort(key=_rect_p)

    # split rect list into two halves (partitions 0-63 / 64-127) of equal
    # padded chunk count
    areas = [h * w for (ya, h, xa, w) in rects]
    total = sum(areas)
    cum, split = 0, len(rects)
    for i, a in enumerate(areas):
        cum += a
        if cum >= total / 2:
            split = i + 1
            break
    halves = [rects[:split], rects[split:]]
    half_used = []
    pix2ring = {}    # (y, x) -> (half, local ring position)
    half_lists = [[], []]
    for hh, rl in enumerate(halves):
        # split the head rects finer so the first chunks land fast (short
        # DMAs) and the PE pipeline starts early
        fine = []
        cum = 0
        for (ya, h, xa, w) in rl:
            if cum < 3 * CHUNK and h > 10:
                for y in range(ya, ya + h, 10):
                    fine.append((y, min(10, ya + h - y), xa, w))
            else:
                fine.append((ya, h, xa, w))
            cum += h * w
        off = 0
        for (ya, h, xa, w) in fine:
            half_lists[hh].append((hh, off, ya, h, xa, w))
            for r in range(h):
                for ccc in range(w):
                    key = (ya + r, xa + ccc)
                    if key not in pix2ring:
                        pix2ring[key] = (hh, off + r * w + ccc)
            off += h * w
        half_used.append(off)
    nchunk_h = (max(half_used) + CHUNK - 1) // CHUNK
    hl = nchunk_h * CHUNK
    # emission order: interleave the two halves so pair inputs arrive together
    order = []
    i0 = i1 = 0
    a0 = a1 = 0
    while i0 < len(half_lists[0]) or i1 < len(half_lists[1]):
        take0 = i1 >= len(half_lists[1]) or (i0 < len(half_lists[0]) and a0 <= a1)
        if take0:
            r = half_lists[0][i0]; i0 += 1; a0 += r[3] * r[5]
        else:
            r = half_lists[1][i1]; i1 += 1; a1 += r[3] * r[5]
        order.append(r)
    rect_dmas = order

    # matmuls: one per (half, chunk, quarter) with hits; interval = min..max p
    hits = {}
    for p, yy, xx, w in corner_list:
        hh, loc = pix2ring[(yy, xx)]
        c = loc // CHUNK
        d = hits.setdefault((hh, c, p // QW), {}).setdefault(p, {})
        r = loc % CHUNK
        d[r] = d.get(r, 0.0) + w

    # group by pair (chunk index), emission order: pair 0..nchunk_h-1
    plan_mms = []    # (pair, half, pst, plen, s_off) in emission order
    s_cols = []      # (hh, c, p, {row: w})
    s_off = 0
    for c in range(nchunk_h):
        for hh in (0, 1):
            for q in range(NQ):
                key = (hh, c, q)
                if key not in hits:
                    continue
                ps = sorted(hits[key])
                pst, pen = ps[0], ps[-1]
                plen = pen - pst + 1
                plan_mms.append((c, hh, pst, plen, s_off))
                for p in range(pst, pen + 1):
                    s_cols.append((hh, c, p, hits[key].get(p, {})))
                s_off += plen
    S = np.zeros((CHUNK, s_off), dtype=np.float32)
    for j, (hh, c, p, rws) in enumerate(s_cols):
        for r, w in rws.items():
            S[r, j] += np.float32(w)

    # coverage check: every output column hit by at least one matmul
    covered = np.zeros(W_C, dtype=bool)
    for (c, hh, pst, plen, so) in plan_mms:
        covered[pst:pst + plen] = True
    assert covered.all(), "some output columns never written"

    # per-quarter last matmul (emission index) for staging triggers
    last_mm_of_quarter = {}
    for i, (c, hh, pst, plen, so) in enumerate(plan_mms):
        for q in range(pst // QW, (pst + plen - 1) // QW + 1):
            last_mm_of_quarter[q] = i

    return dict(rect_dmas=rect_dmas, half_used=half_used, hl=hl,
                nchunk_h=nchunk_h, S=S, plan_mms=plan_mms, sum_m=s_off,
                last_mm_of_quarter=last_mm_of_quarter)


# ------------------------------------------------------- walrus wait-split
def split_waits_json(bir, maxw=1):
    """This neuronxcc walrus accepts at most one sync-wait per instruction;
    move excess waits onto preceding wait-only EventSemaphore ops."""
    uid = [0]
    for fn in bir["functions"]:
        for blk in fn["blocks"]:
            out = []
            for inst in blk["instructions"]:
                si = inst.get("sync_info")
                if si and si.get("on_wait") and len(si["on_wait"]) > maxw:
                    waits = si["on_wait"]
                    extra, keep = waits[:-maxw], waits[-maxw:]
                    for i in range(0, len(extra), maxw):
                        uid[0] += 1
                        out.append({
                            "debug": inst.get("debug", 0),
                            "engine": inst["engine"],
                            "ins": [],
                            "name": f"I-ws-{uid[0]}",
                            "opcode": "EventSemaphore",
                            "outs": [],
                            "sync_info": {"on_update": [],
                                          "on_wait": extra[i:i + maxw]},
                        })
                    si["on_wait"] = keep
                out.append(inst)
            blk["instructions"] = out
    return bir


# ------------------------------------------------------------ device build
def build_nc(plan, repeat=1):
    import concourse.bass as bass
    import concourse.mybir as mybir
    from concourse.tile import TileContext

    class PatchedBass(bass.Bass):
        def to_json_bytes(self):
            data = json.loads(super().to_json_bytes())
            return json.dumps(split_waits_json(data, 1)).encode()

    nc = PatchedBass()
    x_in = nc.dram_tensor("bev", [PLANES, H_B, W_B], mybir.dt.float32,
                          kind="ExternalInput")
    # unused input whose shape varies with `repeat`: defeats the NEFF cache's
    # shape-only HLO hash so timing variants compile separately
    nc.dram_tensor("nonce", [1, max(1, repeat)], mybir.dt.float32,
                   kind="ExternalInput")
    out = nc.dram_tensor("out", [PLANES, H_C, W_C], mybir.dt.float32,
                         kind="ExternalOutput")
    s_const = nc.inline_tensor(plan["S"], name="s_const")
    ident = nc.inline_tensor(np.eye(128, dtype=np.float32), name="ident")

    hl = plan["hl"]
    nch = plan["nchunk_h"]

    with TileContext(nc) as tc:
        with tc.tile_pool(name="sb", bufs=1) as pool, \
             tc.tile_pool(name="pst", bufs=4, space="PSUM") as pst, \
             tc.tile_pool(name="psc", bufs=1, space="PSUM") as psc:
            x_sb = pool.tile([128, hl], mybir.dt.float32)
            xt_sb = pool.tile([128, nch * CHUNK], mybir.dt.float32)
            s_sb = pool.tile([CHUNK, plan["sum_m"]], mybir.dt.float32)
            id_sb = pool.tile([128, 128], mybir.dt.float32)
            col_sb = pool.tile([PLANES, W_C], mybir.dt.float32)
            col_int = pool.tile([128, W_C], mybir.dt.float32)

            nc.gpsimd.dma_start(s_sb[:], s_const[:])
            nc.gpsimd.dma_start(id_sb[:], ident[:])

            for _rep in range(repeat):
                # pad/garbage ring tails must read as zeros
                for hh in (0, 1):
                    used = plan["half_used"][hh]
                    if hl > used:
                        nc.vector.memset(
                            x_sb[64 * hh:64 * hh + 64, used:hl], 0.0)

                # ring cover loads: scalar HWDGE (~3.3ns/descriptor) and
                # gpsimd SWDGE (~1.4ns/descriptor gen-bound) carry the input;
                # sync helps only until ~15us of load so it is free for the
                # output quarters afterwards
                qload = {"sync": 0.0, "scalar": 0.0, "gpsimd": 0.0}
                qeng = {"sync": nc.sync, "scalar": nc.scalar,
                        "gpsimd": nc.gpsimd}
                for (hh, off, ya, h, xa, w) in plan["rect_dmas"]:
                    d = 64.0 * h
                    cost = {"scalar": 650.0 + 3.3 * d,
                            "gpsimd": 1000.0 + 1.8 * d}
                    if qload["sync"] < 20000.0:
                        cost["sync"] = 650.0 + 3.3 * d
                    qn = min(cost, key=lambda q: qload[q] + cost[q])
                    qload[qn] += cost[qn]
                    qeng[qn].dma_start(
                        x_sb[64 * hh:64 * hh + 64, off:off + h * w],
                        x_in[:, ya:ya + h, xa:xa + w])

                # output quarter accumulators: 4 PSUM banks
                col_q = [psc.tile([PLANES, QW], mybir.dt.float32,
                                  name=f"colq{q}", tag=f"colq{q}")
                         for q in range(NQ)]
                for q in range(NQ):
                    nc.vector.memset(col_q[q][:], 0.0)

                # per-pair: one [128,128] transpose covers one chunk of each
                # half; copyback to SBUF; then that pair's matmuls; stage +
                # output DMA fire as quarters complete
                mms = plan["plan_mms"]
                mmi = 0
                stage_by_mm = {}
                for q, i in plan["last_mm_of_quarter"].items():
                    stage_by_mm.setdefault(i, []).append(q)
                quarter_emitted = [False] * NQ
                emitted_t = 0

                def emit_transpose(c):
                    xt_ps = pst.tile([128, 128], mybir.dt.float32,
                                     name=f"xtps{c}", tag="xtps")
                    nc.tensor.transpose(
                        xt_ps[:], x_sb[:, CHUNK * c:CHUNK * (c + 1)], id_sb[:])
                    nc.vector.tensor_copy(
                        xt_sb[:, CHUNK * c:CHUNK * (c + 1)], xt_ps[:])

                for c in range(nch):
                    # keep 2-3 transposes in flight ahead of this pair's
                    # matmuls so PE never stalls on the DVE copyback
                    while emitted_t < min(c + 3, nch):
                        emit_transpose(emitted_t)
                        emitted_t += 1
                    while mmi < len(mms) and mms[mmi][0] == c:
                        _, hh, pst_, plen, so = mms[mmi]
                        q = pst_ // QW
                        nc.tensor.matmul(
                            col_q[q][:, pst_ - QW * q:pst_ - QW * q + plen],
                            xt_sb[:, CHUNK * c + 64 * hh:CHUNK * c + 64 * hh + 64],
                            s_sb[:, so:so + plen],
                            start=False, stop=False, skip_group_check=True)
                        for q in stage_by_mm.get(mmi, []):
                            # stage quarter q: PSUM -> col_sb, duplicate into
                            # (plane, z-half) pairs on 128 partitions, then
                            # one broadcast output DMA with 2KB rows on sync
                            qs = slice(QW * q, QW * (q + 1))
                            nc.vector.tensor_copy(col_sb[:, qs], col_q[q][:])
                            nc.sync.dma_start(col_int[0::2, qs], col_sb[:, qs])
                            nc.sync.dma_start(col_int[1::2, qs], col_sb[:, qs])
                            srcb = col_int[:, None, qs].to_broadcast(
                                (128, H_C // 2, QW))
                            dst = bass.AP(out, QW * q, [
                                [H_C // 2 * W_C, 128],
                                [W_C, H_C // 2],
                                [1, QW],
                            ])
                            nc.sync.dma_start(dst, srcb)
                            quarter_emitted[q] = True
                        mmi += 1
                assert mmi == len(mms) and all(quarter_emitted)
    return nc


# ------------------------------------------------------------------ runner
def _get_state():
    if "state" in _CACHE:
        return _CACHE["state"]
    import jax
    import concourse.mybir as mybir
    from concourse import bass2jax
    from jax.sharding import Mesh, PartitionSpec
    from jax.experimental.shard_map import shard_map

    plan = build_plan()
    nc = build_nc(plan)
    bass2jax.install_neuronx_cc_hook()

    partition_name = (nc.partition_id_tensor.name
                      if nc.partition_id_tensor else None)
    in_names, out_names, out_avals, zero_outs = [], [], [], []
    for alloc in nc.m.functions[0].allocations:
        if not isinstance(alloc, mybir.MemoryLocationSet):
            continue
        name = alloc.memorylocations[0].name
        if alloc.kind == "ExternalInput":
            if name != partition_name:
                in_names.append(name)
        elif alloc.kind == "ExternalOutput":
            shape = tuple(alloc.tensor_shape)
            dtype = mybir.dt.np(alloc.dtype)
            out_names.append(name)
            out_avals.append(jax.core.ShapedArray(shape, dtype))
            zero_outs.append(np.zeros(shape, dtype))
    n_params = len(in_names)
    n_outs = len(out_names)
    all_names = in_names + out_names
    if partition_name is not None:
        all_names = all_names + [partition_name]
    donate = tuple(range(n_params, n_params + n_outs))

    def _body(*args):
        operands = list(args)
        if partition_name is not None:
            operands.append(bass2jax.partition_id_tensor())
        outs = bass2jax._bass_exec_p.bind(
            *operands,
            out_avals=tuple(out_avals),
            in_names=tuple(all_names),
            out_names=tuple(out_names),
            lowering_input_output_aliases=(),
            sim_require_finite=True,
            sim_require_nnan=True,
            nc=nc,
        )
        return tuple(outs)

    devices = jax.devices()[:NCORES]
    mesh = Mesh(np.asarray(devices), ("core",))
    specs = (PartitionSpec("core"),) * (n_params + n_outs)
    out_specs = (PartitionSpec("core"),) * n_outs
    fn = jax.jit(
        shard_map(_body, mesh=mesh, in_specs=specs, out_specs=out_specs,
                  check_rep=False),
        donate_argnums=donate, keep_unused=True)

    nonce = np.zeros((NCORES, 1), np.float32)
    state = dict(fn=fn, zero_outs=zero_outs, prev=None, nc=nc, plan=plan,
                 nonce=nonce)
    _CACHE["state"] = state
    return state


def kernel(bev_feat):
    bev = np.ascontiguousarray(np.asarray(bev_feat, dtype=np.float32))
    st = _get_state()
    global_in = bev.reshape(B * C, H_B, W_B)  # cores split axis 0: 64 each
    if st["prev"] is not None:
        zouts = st["prev"]          # donate previous device outputs
    else:
        zouts = [np.zeros((NCORES * z.shape[0], *z.shape[1:]), z.dtype)
                 for z in st["zero_outs"]]
    outs = st["fn"](global_in, st["nonce"], *zouts)
    result = np.asarray(outs[0])    # [512, 64, 2048]
    st["prev"] = list(outs)
    return result.reshape(B, C, H_C, W_C)


# revision 17
# speedup vs baseline: 1.6267x; 1.0913x over previous
"""BEV->Cylinder bilinear ring-sampling kernel for 8 Trainium2 NeuronCores.

Strategy (per core, 64 of the 512 (b,c) planes):
  * The 2048 sample points lie on a circle of radius 255.5 px; a tight
    rectangle cover of that ring (~17K px, tall-narrow rects on the steep
    arcs, short-wide on the shallow ones) is DMA'd into SBUF as
    X[plane, ring_px], rects split across the sync/scalar HWDGE queues and
    the gpsimd SWDGE queue, in circle order with the two ring halves
    interleaved.
  * The ring is split into two halves of equal chunk count living on SBUF
    partitions 0-63 / 64-127.  One PE transpose per 128-px column window
    transposes BOTH halves' chunks at once ([128,128] -> [128,128]).
  * col[plane, p] = sum_k w_k * I[corner_k(p)] accumulates in 4 PSUM
    quarter banks via one PE matmul per (chunk, output-eighth):
    col[:, win] += XT_chunk.T @ S_chunk_win with a host-precomputed
    sparse-in-dense weight matrix S.
  * As each output eighth's accumulation completes it is copied to SBUF and
    broadcast over the 64 z-bins by a single output DMA (2048-row, 1KB-row
    descriptors), eighths alternating between the two HWDGE queues so
    output write bandwidth is busy from ~15us onward.
All geometry/weights are input-independent compile-time constants baked into
the NEFF. Input dtype f32 is preserved end to end.
"""
import json
import math

import numpy as np

B, C, H_B, W_B = 4, 128, 512, 512
H_C, W_C = 64, 2048
MAX_RANGE = 50.0
XMIN, XMAX, YMIN, YMAX = -50.0, 50.0, -50.0, 50.0
NCORES = 8
PLANES = B * C // NCORES  # 64 planes per core

CHUNK = 128       # ring pixels per PE-transpose column window
NQ = 4            # output quarters (2KB DMA rows — the output sweet spot)
QW = W_C // NQ    # 512 columns per quarter
W_NARROW = 16     # row-merge width cap for narrow rects
SLACK = 6         # extra width allowed when merging rows
CLUSTER_GAP = 16  # split row runs when x-gap exceeds this

_CACHE = {}


# ----------------------------------------------------------------- geometry
def _sample_xy():
    """Sampling pixel coords exactly like the reference (jnp on CPU);
    numpy fallback differs only at ULP level."""
    try:
        import jax
        import jax.numpy as jnp
        cpu = jax.devices("cpu")[0]
        with jax.default_device(cpu):
            phi = jnp.linspace(-math.pi, math.pi, W_C)
            x_g = MAX_RANGE * jnp.cos(phi)
            y_g = MAX_RANGE * jnp.sin(phi)
            x = (x_g - XMIN) / (XMAX - XMIN) * (W_B - 1)
            y = (YMAX - y_g) / (YMAX - YMIN) * (H_B - 1)
            return np.asarray(x, np.float32), np.asarray(y, np.float32)
    except Exception:
        phi = np.linspace(-math.pi, math.pi, W_C, dtype=np.float32)
        x_g = (MAX_RANGE * np.cos(phi)).astype(np.float32)
        y_g = (MAX_RANGE * np.sin(phi)).astype(np.float32)
        x = ((x_g - XMIN) / (XMAX - XMIN) * (W_B - 1)).astype(np.float32)
        y = ((YMAX - y_g) / (YMAX - YMIN) * (H_B - 1)).astype(np.float32)
        return x, y


def _corners():
    x, y = _sample_xy()
    x0 = np.floor(x).astype(np.int64)
    y0 = np.floor(y).astype(np.int64)
    wx1 = (x - x0.astype(np.float32)).astype(np.float32)
    wx0 = (np.float32(1.0) - wx1).astype(np.float32)
    wy1 = (y - y0.astype(np.float32)).astype(np.float32)
    wy0 = (np.float32(1.0) - wy1).astype(np.float32)
    out = []
    for xi, wx in ((x0, wx0), (x0 + 1, wx1)):
        for yi, wy in ((y0, wy0), (y0 + 1, wy1)):
            w = (wx * wy).astype(np.float32)
            valid = (xi >= 0) & (xi < W_B) & (yi >= 0) & (yi < H_B)
            for p in range(W_C):
                if valid[p]:
                    out.append((p, int(yi[p]), int(xi[p]), float(w[p])))
    return out


def _build_cover(corner_list):
    """Tight rect cover of the ring: per-image-row x-runs merged vertically
    while the union width stays narrow. Steep arcs become tall-narrow rects,
    shallow arcs short-wide ones."""
    rows = {}
    for p, yy, xx, w in corner_list:
        rows.setdefault(yy, set()).add(xx)
    row_clusters = {}
    for yy, xs in rows.items():
        xs = sorted(xs)
        cl = [[xs[0], xs[0]]]
        for xx in xs[1:]:
            if xx - cl[-1][1] > CLUSTER_GAP:
                cl.append([xx, xx])
            else:
                cl[-1][1] = xx
        row_clusters[yy] = cl
    out = []
    open_rects = {}  # side -> [ya, yb, xa, xb, max_single_row_w]

    def flush(side):
        if side in open_rects:
            r = open_rects.pop(side)
            out.append((r[0], r[1] - r[0], r[2], r[3] - r[2] + 1))

    for yy in range(H_B):
        cl = row_clusters.get(yy)
        if cl is None:
            flush('L'); flush('R')
            continue
        if len(cl) == 1:
            flush('R')
            assign = [('L', cl[0])]
        else:
            assign = [('L', cl[0]), ('R', cl[-1])]
            for c in cl[1:-1]:
                out.append((yy, 1, c[0], c[1] - c[0] + 1))
        for side, (xa, xb) in assign:
            w = xb - xa + 1
            if side in open_rects:
                r = open_rects[side]
                nxa, nxb = min(r[2], xa), max(r[3], xb)
                lim = max(W_NARROW, int(1.15 * max(r[4], w)) + SLACK)
                if nxb - nxa + 1 <= lim:
                    r[1] = yy + 1
                    r[2], r[3] = nxa, nxb
                    r[4] = max(r[4], w)
                    continue
                flush(side)
            open_rects[side] = [yy, yy + 1, xa, xb, w]
    flush('L'); flush('R')
    return out


def build_plan():
    corner_list = _corners()
    rects = _build_cover(corner_list)
    # split very tall rects so the half alternation can balance
    split_rects = []
    for (ya, h, xa, w) in rects:
        nparts = (h + 95) // 96
        step = (h + nparts - 1) // nparts
        for y in range(ya, ya + h, step):
            split_rects.append((y, min(step, ya + h - y), xa, w))
    rects = split_rects

    # order rects along the sampling circle; assign each to an eighth-arc.
    # half0 carries the even eighth arcs (e0 e2 e4 e6), half1 the odd ones:
    # output quarter q is then complete after (q+1)/4 of the pair stream,
    # so output DMAs start early and stream continuously
    def _rect_p(rc):
        ya, h, xa, w = rc
        yc, xc = ya + h / 2.0, xa + w / 2.0
        phi = math.atan2(255.5 - yc, xc - 255.5)
        f = (phi + math.pi) / (2 * math.pi)
        # seam rects (p wraps 2047->0 at the left edge) must load FIRST so
        # quarter 0 can start early; quarter 3 also needs them but its last
        # matmul comes from its late-arc chunks anyway
        return f - 1.0 if f > 0.97 else f
    rects.sort(key=_rect_p)
    # within each quarter-arc, alternate rects between the two halves by
    # cumulative area: segments stay aligned and areas balance
    halves = [[], []]
    seg_area = [[0] * NQ, [0] * NQ]
    for rc in rects:
        f = _rect_p(rc)
        qa = min(NQ - 1, max(0, int((f if f > 0 else 0.0) * NQ)))
        hh = 0 if seg_area[0][qa] <= seg_area[1][qa] else 1
        seg_area[hh][qa] += rc[1] * rc[3]
        halves[hh].append(rc)
    half_used = []
    pix2ring = {}    # (y, x) -> (half, local ring position)
    half_lists = [[], []]
    for hh, rl in enumerate(halves):
        # split the head rects finer so the first chunks land fast (short
        # DMAs) and the PE pipeline starts early
        fine = []
        cum = 0
        for (ya, h, xa, w) in rl:
            if cum < 2 * CHUNK and h > 12:
                for y in range(ya, ya + h, 12):
                    fine.append((y, min(12, ya + h - y), xa, w))
            else:
                fine.append((ya, h, xa, w))
            cum += h * w
        off = 0
        for (ya, h, xa, w) in fine:
            half_lists[hh].append((hh, off, ya, h, xa, w))
            for r in range(h):
                for ccc in range(w):
                    key = (ya + r, xa + ccc)
                    if key not in pix2ring:
                        pix2ring[key] = (hh, off + r * w + ccc)
            off += h * w
        half_used.append(off)
    nchunk_h = (max(half_used) + CHUNK - 1) // CHUNK
    hl = nchunk_h * CHUNK
    # emission order: interleave the two halves so pair inputs arrive together
    order = []
    i0 = i1 = 0
    a0 = a1 = 0
    while i0 < len(half_lists[0]) or i1 < len(half_lists[1]):
        take0 = i1 >= len(half_lists[1]) or (i0 < len(half_lists[0]) and a0 <= a1)
        if take0:
            r = half_lists[0][i0]; i0 += 1; a0 += r[3] * r[5]
        else:
            r = half_lists[1][i1]; i1 += 1; a1 += r[3] * r[5]
        order.append(r)
    rect_dmas = order

    # matmuls: one per (half, chunk, quarter) with hits; interval = min..max p
    hits = {}
    for p, yy, xx, w in corner_list:
        hh, loc = pix2ring[(yy, xx)]
        c = loc // CHUNK
        d = hits.setdefault((hh, c, p // QW), {}).setdefault(p, {})
        r = loc % CHUNK
        d[r] = d.get(r, 0.0) + w

    # group by pair (chunk index), emission order: pair 0..nchunk_h-1
    plan_mms = []    # (pair, half, pst, plen, s_off) in emission order
    s_cols = []      # (hh, c, p, {row: w})
    s_off = 0
    for c in range(nchunk_h):
        for hh in (0, 1):
            for q in range(NQ):
                key = (hh, c, q)
                if key not in hits:
                    continue
                ps = sorted(hits[key])
                pst, pen = ps[0], ps[-1]
                plen = pen - pst + 1
                plan_mms.append((c, hh, pst, plen, s_off))
                for p in range(pst, pen + 1):
                    s_cols.append((hh, c, p, hits[key].get(p, {})))
                s_off += plen
    S = np.zeros((CHUNK, s_off), dtype=np.float32)
    for j, (hh, c, p, rws) in enumerate(s_cols):
        for r, w in rws.items():
            S[r, j] += np.float32(w)

    # coverage check: every output column hit by at least one matmul
    covered = np.zeros(W_C, dtype=bool)
    for (c, hh, pst, plen, so) in plan_mms:
        covered[pst:pst + plen] = True
    assert covered.all(), "some output columns never written"

    # per-quarter last matmul (emission index) for staging triggers
    last_mm_of_quarter = {}
    for i, (c, hh, pst, plen, so) in enumerate(plan_mms):
        for q in range(pst // QW, (pst + plen - 1) // QW + 1):
            last_mm_of_quarter[q] = i

    return dict(rect_dmas=rect_dmas, half_used=half_used, hl=hl,
                nchunk_h=nchunk_h, S=S, plan_mms=plan_mms, sum_m=s_off,
                last_mm_of_quarter=last_mm_of_quarter)


# ------------------------------------------------------- walrus wait-split
def split_waits_json(bir, maxw=1):
    """This neuronxcc walrus accepts at most one sync-wait per instruction;
    move excess waits onto preceding wait-only EventSemaphore ops."""
    uid = [0]
    for fn in bir["functions"]:
        for blk in fn["blocks"]:
            out = []
            for inst in blk["instructions"]:
                si = inst.get("sync_info")
                if si and si.get("on_wait") and len(si["on_wait"]) > maxw:
                    waits = si["on_wait"]
                    extra, keep = waits[:-maxw], waits[-maxw:]
                    for i in range(0, len(extra), maxw):
                        uid[0] += 1
                        out.append({
                            "debug": inst.get("debug", 0),
                            "engine": inst["engine"],
                            "ins": [],
                            "name": f"I-ws-{uid[0]}",
                            "opcode": "EventSemaphore",
                            "outs": [],
                            "sync_info": {"on_update": [],
                                          "on_wait": extra[i:i + maxw]},
                        })
                    si["on_wait"] = keep
                out.append(inst)
            blk["instructions"] = out
    return bir


# ------------------------------------------------------------ device build
def build_nc(plan, repeat=1):
    import concourse.bass as bass
    import concourse.mybir as mybir
    from concourse.tile import TileContext

    class PatchedBass(bass.Bass):
        def to_json_bytes(self):
            data = json.loads(super().to_json_bytes())
            return json.dumps(split_waits_json(data, 1)).encode()

    nc = PatchedBass()
    x_in = nc.dram_tensor("bev", [PLANES, H_B, W_B], mybir.dt.float32,
                          kind="ExternalInput")
    # unused input whose shape varies with `repeat`: defeats the NEFF cache's
    # shape-only HLO hash so timing variants compile separately
    nc.dram_tensor("nonce", [1, max(1, repeat)], mybir.dt.float32,
                   kind="ExternalInput")
    out = nc.dram_tensor("out", [PLANES, H_C, W_C], mybir.dt.float32,
                         kind="ExternalOutput")
    s_const = nc.inline_tensor(plan["S"], name="s_const")
    ident = nc.inline_tensor(np.eye(128, dtype=np.float32), name="ident")

    hl = plan["hl"]
    nch = plan["nchunk_h"]

    with TileContext(nc) as tc:
        with tc.tile_pool(name="sb", bufs=1) as pool, \
             tc.tile_pool(name="pst", bufs=4, space="PSUM") as pst, \
             tc.tile_pool(name="psc", bufs=1, space="PSUM") as psc:
            x_sb = pool.tile([128, hl], mybir.dt.float32)
            xt_sb = pool.tile([128, nch * CHUNK], mybir.dt.float32)
            s_sb = pool.tile([CHUNK, plan["sum_m"]], mybir.dt.float32)
            id_sb = pool.tile([128, 128], mybir.dt.float32)
            col_sb = pool.tile([PLANES, W_C], mybir.dt.float32)
            col_int = pool.tile([128, W_C], mybir.dt.float32)

            nc.gpsimd.dma_start(s_sb[:], s_const[:])
            nc.gpsimd.dma_start(id_sb[:], ident[:])

            for _rep in range(repeat):
                # pad/garbage ring tails must read as zeros
                for hh in (0, 1):
                    used = plan["half_used"][hh]
                    if hl > used:
                        nc.vector.memset(
                            x_sb[64 * hh:64 * hh + 64, used:hl], 0.0)

                # ring cover loads: scalar HWDGE (~3.3ns/descriptor) and
                # gpsimd SWDGE (~1.4ns/descriptor gen-bound) carry the input;
                # sync helps only until ~15us of load so it is free for the
                # output quarters afterwards
                qload = {"sync": 0.0, "scalar": 0.0, "gpsimd": 0.0}
                qeng = {"sync": nc.sync, "scalar": nc.scalar,
                        "gpsimd": nc.gpsimd}
                for (hh, off, ya, h, xa, w) in plan["rect_dmas"]:
                    d = 64.0 * h
                    cost = {"gpsimd": 1000.0 + 1.4 * d}
                    if qload["scalar"] < 45000.0:
                        cost["scalar"] = 650.0 + 2.6 * d
                    if qload["sync"] < 12000.0:
                        cost["sync"] = 650.0 + 2.6 * d
                    qn = min(cost, key=lambda q: qload[q] + cost[q])
                    qload[qn] += cost[qn]
                    qeng[qn].dma_start(
                        x_sb[64 * hh:64 * hh + 64, off:off + h * w],
                        x_in[:, ya:ya + h, xa:xa + w])

                # output quarter accumulators: 4 PSUM banks
                col_q = [psc.tile([PLANES, QW], mybir.dt.float32,
                                  name=f"colq{q}", tag=f"colq{q}")
                         for q in range(NQ)]
                for q in range(NQ):
                    nc.vector.memset(col_q[q][:], 0.0)

                # per-pair: one [128,128] transpose covers one chunk of each
                # half; copyback to SBUF; then that pair's matmuls; stage +
                # output DMA fire as quarters complete
                mms = plan["plan_mms"]
                mmi = 0
                stage_by_mm = {}
                for q, i in plan["last_mm_of_quarter"].items():
                    stage_by_mm.setdefault(i, []).append(q)
                quarter_emitted = [False] * NQ
                emitted_t = 0

                def emit_transpose(c):
                    xt_ps = pst.tile([128, 128], mybir.dt.float32,
                                     name=f"xtps{c}", tag="xtps")
                    nc.tensor.transpose(
                        xt_ps[:], x_sb[:, CHUNK * c:CHUNK * (c + 1)], id_sb[:])
                    nc.vector.tensor_copy(
                        xt_sb[:, CHUNK * c:CHUNK * (c + 1)], xt_ps[:])

                for c in range(nch):
                    # keep 2-3 transposes in flight ahead of this pair's
                    # matmuls so PE never stalls on the DVE copyback
                    while emitted_t < min(c + 3, nch):
                        emit_transpose(emitted_t)
                        emitted_t += 1
                    while mmi < len(mms) and mms[mmi][0] == c:
                        _, hh, pst_, plen, so = mms[mmi]
                        q = pst_ // QW
                        nc.tensor.matmul(
                            col_q[q][:, pst_ - QW * q:pst_ - QW * q + plen],
                            xt_sb[:, CHUNK * c + 64 * hh:CHUNK * c + 64 * hh + 64],
                            s_sb[:, so:so + plen],
                            start=False, stop=False, skip_group_check=True)
                        for q in stage_by_mm.get(mmi, []):
                            # stage quarter q: PSUM -> col_sb, duplicate into
                            # (plane, z-half) pairs on 128 partitions, then
                            # one broadcast output DMA with 2KB rows; even
                            # quarters on sync, odd on scalar
                            oq = nc.sync if q % 2 == 0 else nc.scalar
                            qs = slice(QW * q, QW * (q + 1))
                            nc.vector.tensor_copy(col_sb[:, qs], col_q[q][:])
                            oq.dma_start(col_int[0::2, qs], col_sb[:, qs])
                            oq.dma_start(col_int[1::2, qs], col_sb[:, qs])
                            srcb = col_int[:, None, qs].to_broadcast(
                                (128, H_C // 2, QW))
                            dst = bass.AP(out, QW * q, [
                                [H_C // 2 * W_C, 128],
                                [W_C, H_C // 2],
                                [1, QW],
                            ])
                            oq.dma_start(dst, srcb)
                            quarter_emitted[q] = True
                        mmi += 1
                assert mmi == len(mms) and all(quarter_emitted)
    return nc


# ------------------------------------------------------------------ runner
def _get_state():
    if "state" in _CACHE:
        return _CACHE["state"]
    import jax
    import concourse.mybir as mybir
    from concourse import bass2jax
    from jax.sharding import Mesh, PartitionSpec
    from jax.experimental.shard_map import shard_map

    plan = build_plan()
    nc = build_nc(plan)
    bass2jax.install_neuronx_cc_hook()

    partition_name = (nc.partition_id_tensor.name
                      if nc.partition_id_tensor else None)
    in_names, out_names, out_avals, zero_outs = [], [], [], []
    for alloc in nc.m.functions[0].allocations:
        if not isinstance(alloc, mybir.MemoryLocationSet):
            continue
        name = alloc.memorylocations[0].name
        if alloc.kind == "ExternalInput":
            if name != partition_name:
                in_names.append(name)
        elif alloc.kind == "ExternalOutput":
            shape = tuple(alloc.tensor_shape)
            dtype = mybir.dt.np(alloc.dtype)
            out_names.append(name)
            out_avals.append(jax.core.ShapedArray(shape, dtype))
            zero_outs.append(np.zeros(shape, dtype))
    n_params = len(in_names)
    n_outs = len(out_names)
    all_names = in_names + out_names
    if partition_name is not None:
        all_names = all_names + [partition_name]
    donate = tuple(range(n_params, n_params + n_outs))

    def _body(*args):
        operands = list(args)
        if partition_name is not None:
            operands.append(bass2jax.partition_id_tensor())
        outs = bass2jax._bass_exec_p.bind(
            *operands,
            out_avals=tuple(out_avals),
            in_names=tuple(all_names),
            out_names=tuple(out_names),
            lowering_input_output_aliases=(),
            sim_require_finite=True,
            sim_require_nnan=True,
            nc=nc,
        )
        return tuple(outs)

    devices = jax.devices()[:NCORES]
    mesh = Mesh(np.asarray(devices), ("core",))
    specs = (PartitionSpec("core"),) * (n_params + n_outs)
    out_specs = (PartitionSpec("core"),) * n_outs
    fn = jax.jit(
        shard_map(_body, mesh=mesh, in_specs=specs, out_specs=out_specs,
                  check_rep=False),
        donate_argnums=donate, keep_unused=True)

    nonce = np.zeros((NCORES, 1), np.float32)
    state = dict(fn=fn, zero_outs=zero_outs, prev=None, nc=nc, plan=plan,
                 nonce=nonce)
    _CACHE["state"] = state
    return state


def kernel(bev_feat):
    bev = np.ascontiguousarray(np.asarray(bev_feat, dtype=np.float32))
    st = _get_state()
    global_in = bev.reshape(B * C, H_B, W_B)  # cores split axis 0: 64 each
    if st["prev"] is not None:
        zouts = st["prev"]          # donate previous device outputs
    else:
        zouts = [np.zeros((NCORES * z.shape[0], *z.shape[1:]), z.dtype)
                 for z in st["zero_outs"]]
    outs = st["fn"](global_in, st["nonce"], *zouts)
    result = np.asarray(outs[0])    # [512, 64, 2048]
    st["prev"] = list(outs)
    return result.reshape(B, C, H_C, W_C)
